# revision 15
# baseline (speedup 1.0000x reference)
"""AttentionBlock (GroupNorm -> qkv -> MHA -> proj -> residual) on 8 trn2 cores.

Data-parallel over batch: 16 batches -> 2 per core. No collectives.

Per-core math (per batch item, c=512 channels, hw=1024 spatial, 8 heads x 64):
  xn = groupnorm(x)                     [c, hw] layout (c on partitions)
  q,k = Wqk^T.T @ xn + b                [2c, hw]
  vT  = xn.T @ WvT + bv (broadcast)     [hw, c]   (direct transposed matmul)
  per head: S^T = k^T q                 [s=hw, t=hw]   (d=64 contraction)
            P = exp(S^T / 8)            (softmax w/o max-sub; logits ~N(0,1))
            AV: lhsT=[vT_h | ones] -> rows 0..64 unnormalized out, row 64 = r
            h = AV[0:64] * (1/r)
  y = x + WprojT.T @ h + proj_b

Rev B over the 417us baseline:
  - All startup DMAs batched (1 for x per batch, 1 packed const, 1 packed
    weight) -- the old kernel spent ~37us just issuing ~60 dma_starts.
  - softmax exp split across engines: head j=0 of each pair on ScalarE
    (table exp), j=1 on VectorE via a one-op Schraudolph exp that writes
    bf16 *bit patterns* through an int16 convert (err ~2% on pexp, which
    mostly cancels in p/r; ~1e-3 on the final output).  ACT was a 147us
    serial bottleneck; now ~93us ACT / ~85us DVE run in parallel under the
    PE's ~139us.
  - q/k bias+copy moved to ScalarE (idle during non-pair phases).
  - PSUM->SBUF r/hu drains on the otherwise idle GpSimd engine.
  - pair rounds emit PE work as [S(st+1,j0) S(st+1,j1) AV(st,j0) AV(st,j1)]
    so the PE queue never head-of-line blocks on the exp chain; qk/vt/proj
    fill the inter-pair gaps (PSUM: 2x[128,1024] S slots + 2 AV accums = 8
    banks, so fillers can't run inside rounds).
  - normalize: per-pair (not per-head) DMA transpose of r, reciprocal on 128
    lanes, bf16 broadcast (256KB/pair instead of 512KB/head).
"""

import os

import numpy as np
import ml_dtypes

import concourse.bass as bass
import concourse.tile as tile
import concourse.mybir as mybir
from concourse import bacc

NUM_HEADS = 8
NUM_GROUPS = 32
EPS = 1e-5
B, C, H, W = 16, 512, 32, 32
HW = H * W                  # 1024
NCORES = 8
BPC = B // NCORES           # 2 batches per core
HD = C // NUM_HEADS         # 64
GS = C // NUM_GROUPS        # 16 channels per group
CT = C // 128               # 4 channel tiles
QKT = 2 * C // 128          # 8 q+k output tiles
ST = HW // 128              # 8 sequence tiles
NH = HW // 512              # 2 moving-dim chunks of 512

F32 = mybir.dt.float32
BF16 = mybir.dt.bfloat16
I16 = mybir.dt.int16
U32 = mybir.dt.uint32
ALU = mybir.AluOpType
ACTF = mybir.ActivationFunctionType

USE_TP = os.environ.get("KERNEL_NO_TP") != "1"
# softmax exp for head j=1 of each pair on DVE (Schraudolph) instead of ACT
USE_SCHR = os.environ.get("KERNEL_NO_SCHR") != "1"
# r/hu PSUM drains on gpsimd
USE_GP = os.environ.get("KERNEL_NO_GP") != "1"
# q/k bias+copy on ScalarE
USE_QT_ACT = os.environ.get("KERNEL_QT_ACT") == "1"

# Schraudolph exp in bf16-bit space: bits = int16(A*x + Bc); bf16 = bits
# computes exp(x/8) for raw logits x.  A = 2^7/(8 ln2).  C=5.0 tuned for
# round-to-nearest convert (max rel ~3.6%, mean +1.3%).
SCHR_A = 128.0 / (8.0 * np.log(2.0))
SCHR_B = 127.0 * 128.0 - 5.0

# packed const layout (f32 columns)
CP_GM = 0            # gm: [128, 4*32]   (group one-hot, per ct)
CP_QKVB = 128        # qkvb: [128, 8]    (bias col per qk out tile)
CP_PROJB = 136       # projb: [128, 4]
CP_GNG = 140         # gng: [128, 4]
CP_GNB = 144         # gnb: [128, 4]
CP_VB = 148          # vbias broadcast: [128, 8*64]
CP_EM = 660          # em: [32, 4*128]   (rows 0:32; transpose of gm)
CP_COLS = 660 + 4 * 128

WQK_W = 3 * C        # 1536 qkv cols per kt chunk
WP_OFF = 3 * C       # proj cols start
WPACK_COLS = 3 * C + C  # 2048


def build(num_devices=NCORES, q_bias=False, v_bias=False, p_bias=False):
    nc = bacc.Bacc("TRN2", target_bir_lowering=False, debug=False,
                   num_devices=num_devices)

    x_d = nc.dram_tensor("x", [BPC, C, HW], F32, kind="ExternalInput").ap()
    wpack_d = nc.dram_tensor("wpack", [128, CT, WPACK_COLS], BF16,
                             kind="ExternalInput").ap()
    cpack_d = nc.dram_tensor("cpack", [128, CP_COLS], F32,
                             kind="ExternalInput").ap()
    out_d = nc.dram_tensor("out", [BPC, C, HW], F32, kind="ExternalOutput").ap()

    with tile.TileContext(nc) as tc:
        _body(tc, nc, x_d, wpack_d, cpack_d, out_d, q_bias, v_bias, p_bias)
    nc.compile()
    return nc


def _body(tc, nc, x_d, wpack_d, cpack_d, out_d, q_bias, v_bias, p_bias):
    from contextlib import ExitStack
    ctx = ExitStack()
    with ctx:
        const = ctx.enter_context(tc.tile_pool(name="const", bufs=1))
        xpool = ctx.enter_context(tc.tile_pool(name="xpool", bufs=2))
        xnpool = ctx.enter_context(tc.tile_pool(name="xnpool", bufs=2 * CT))
        qkvpool = ctx.enter_context(tc.tile_pool(name="qkvpool", bufs=2 * QKT))
        vtapool = ctx.enter_context(tc.tile_pool(name="vtapool", bufs=2 * ST))
        exppool = ctx.enter_context(tc.tile_pool(name="exppool", bufs=6))
        hpool = ctx.enter_context(tc.tile_pool(name="hpool", bufs=2 * CT))
        hupool = ctx.enter_context(tc.tile_pool(name="hupool", bufs=4))
        rbpool = ctx.enter_context(tc.tile_pool(name="rbpool", bufs=2))
        ypool = ctx.enter_context(tc.tile_pool(name="ypool", bufs=2))
        smalls = ctx.enter_context(tc.tile_pool(name="smalls", bufs=6))
        rsm = ctx.enter_context(tc.tile_pool(name="rsm", bufs=1))
        drams = ctx.enter_context(tc.tile_pool(name="drams", bufs=4, space="DRAM"))
        ps_a = ctx.enter_context(tc.tile_pool(name="ps_a", bufs=2, space="PSUM"))
        ps_av = ctx.enter_context(tc.tile_pool(name="ps_av", bufs=2, space="PSUM"))

        # ---- batched input DMAs (4 dma_starts total) ----
        x_sb = [xpool.tile([128, CT, HW], F32, tag="x", name=f"x_{b}")
                for b in range(BPC)]
        cp = const.tile([128, CP_COLS], F32)
        wp = const.tile([128, CT, WPACK_COLS], BF16)

        def dma_x(b):
            # per-ct chunks so gn(b) statistics start on the first 512KB
            for ct in range(CT):
                nc.sync.dma_start(out=x_sb[b][:, ct, :],
                                  in_=x_d[b, ct * 128:(ct + 1) * 128, :])

        nc.sync.dma_start(out=cp, in_=cpack_d)
        dma_x(0)
        # v-cols first (vt runs first), then q/k, then proj
        nc.sync.dma_start(out=wp[:, :, 2 * C:3 * C], in_=wpack_d[:, :, 2 * C:3 * C])
        nc.sync.dma_start(out=wp[:, :, 0:2 * C], in_=wpack_d[:, :, 0:2 * C])
        nc.sync.dma_start(out=wp[:, :, WP_OFF:], in_=wpack_d[:, :, WP_OFF:])
        dma_x(1)

        magic = const.tile([NUM_GROUPS, 1], U32)
        nc.vector.memset(magic, 0x5F3759DF)
        ones64 = const.tile([1, HD], F32)
        nc.vector.memset(ones64, 1.0)

        def wq_ap(kt, c0, c1):
            return wp[:, kt, c0:c1]

        def wproj_ap(kt, ot):
            return wp[:, kt, WP_OFF + ot * 128:WP_OFF + (ot + 1) * 128]

        state = [dict() for _ in range(BPC)]

        def emit_gn(b):
            """group-norm of x_sb[b] -> xns (bf16).  rstd via DVE quake+Newton
            so ScalarE only ever holds the exp table set."""
            s = state[b]
            cm2s, xns = [], []
            for ct in range(CT):
                stats = smalls.tile([128, 2, 6], F32, tag="bnst", name=f"bnst_{b}_{ct}")
                for sg in range(2):
                    nc.vector.bn_stats(out=stats[:, sg, :],
                                       in_=x_sb[b][:, ct, sg * 512:(sg + 1) * 512])
                cmv = smalls.tile([128, 2], F32, tag="cmv", name=f"cmv_{b}_{ct}")
                nc.vector.bn_aggr(out=cmv, in_=stats)
                cm2 = smalls.tile([128, 2], F32, tag="cm2", name=f"cm2_{b}_{ct}")
                nc.vector.tensor_copy(out=cm2[:, 0:1], in_=cmv[:, 0:1])
                nc.vector.tensor_tensor(out=cm2[:, 1:2], in0=cmv[:, 0:1], in1=cmv[:, 0:1], op=ALU.mult)
                nc.vector.tensor_tensor(out=cm2[:, 1:2], in0=cm2[:, 1:2], in1=cmv[:, 1:2], op=ALU.add)
                cm2s.append(cm2)
                xn = xnpool.tile([128, HW], BF16, tag="xn", name=f"xn_{b}_{ct}")
                xns.append(xn)
            ps_g = ps_a.tile([128, HW], F32, tag="psa", name=f"psg_{b}")
            for ct in range(CT):
                nc.tensor.matmul(ps_g[0:NUM_GROUPS, 0:2],
                                 lhsT=cp[:, CP_GM + ct * 32:CP_GM + (ct + 1) * 32],
                                 rhs=cm2s[ct], start=(ct == 0), stop=(ct == CT - 1))
            gstat = smalls.tile([NUM_GROUPS, 2], F32, tag="gstat", name=f"gstat_{b}")
            nc.vector.tensor_scalar_mul(out=gstat, in0=ps_g[0:NUM_GROUPS, 0:2], scalar1=1.0 / GS)
            var_g = smalls.tile([NUM_GROUPS, 1], F32, tag="varg", name=f"varg_{b}")
            nc.vector.tensor_tensor(out=var_g, in0=gstat[:, 0:1], in1=gstat[:, 0:1], op=ALU.mult)
            nc.vector.tensor_tensor(out=var_g, in0=gstat[:, 1:2], in1=var_g, op=ALU.subtract)
            nc.vector.tensor_scalar_add(out=var_g, in0=var_g, scalar1=EPS)
            y_n = smalls.tile([NUM_GROUPS, 1], F32, tag="yn", name=f"yn_{b}")
            t_n = smalls.tile([NUM_GROUPS, 1], F32, tag="tn", name=f"tn_{b}")
            nc.vector.tensor_scalar(out=y_n.bitcast(U32), in0=var_g.bitcast(U32),
                                    scalar1=1, scalar2=None, op0=ALU.logical_shift_right)
            nc.vector.tensor_tensor(out=y_n.bitcast(U32), in0=magic,
                                    in1=y_n.bitcast(U32), op=ALU.subtract)
            for _ in range(3):
                nc.vector.tensor_tensor(out=t_n, in0=var_g, in1=y_n, op=ALU.mult)
                nc.vector.tensor_tensor(out=t_n, in0=t_n, in1=y_n, op=ALU.mult)
                nc.vector.tensor_scalar(out=t_n, in0=t_n, scalar1=-0.5, scalar2=1.5,
                                        op0=ALU.mult, op1=ALU.add)
                nc.vector.tensor_tensor(out=y_n, in0=y_n, in1=t_n, op=ALU.mult)
            nc.vector.tensor_copy(out=gstat[:, 1:2], in_=y_n)
            for ct in range(CT):
                ps_e = ps_a.tile([128, HW], F32, tag="psa", name=f"pse_{b}_{ct}")
                nc.tensor.matmul(ps_e[:, 0:2],
                                 lhsT=cp[0:32, CP_EM + ct * 128:CP_EM + (ct + 1) * 128],
                                 rhs=gstat, start=True, stop=True)
                sc = smalls.tile([128, 1], F32, tag="sc", name=f"sc_{b}_{ct}")
                bi = smalls.tile([128, 1], F32, tag="bi", name=f"bi_{b}_{ct}")
                nc.vector.tensor_tensor(out=sc, in0=cp[:, CP_GNG + ct:CP_GNG + ct + 1],
                                        in1=ps_e[:, 1:2], op=ALU.mult)
                nc.vector.tensor_tensor(out=bi, in0=ps_e[:, 0:1], in1=sc, op=ALU.mult)
                nc.vector.tensor_tensor(out=bi, in0=cp[:, CP_GNB + ct:CP_GNB + ct + 1],
                                        in1=bi, op=ALU.subtract)
                xeng = nc.gpsimd if USE_GP else nc.vector
                xeng.tensor_scalar(out=xns[ct], in0=x_sb[b][:, ct, :],
                                   scalar1=sc, scalar2=bi, op0=ALU.mult, op1=ALU.add)
            s["xns"] = xns

        def emit_vt(b, sts):
            """vT[s, o] = sum_c xn[c, s] * WvT[c, o]; +bias col65=1 for rowsum."""
            s = state[b]
            vtas = s.setdefault("vtas", [None] * ST)
            for st in sts:
                ps_v = ps_a.tile([128, C], F32, tag="psa", name=f"psv_{b}_{st}")
                for kt in range(CT):
                    nc.tensor.matmul(ps_v[:, 0:C],
                                     lhsT=s["xns"][kt][:, st * 128:(st + 1) * 128],
                                     rhs=wq_ap(kt, 2 * C, 3 * C),
                                     start=(kt == 0), stop=(kt == CT - 1))
                vta = vtapool.tile([128, NUM_HEADS, HD + 1], BF16, tag="vta",
                                   name=f"vta_{b}_{st}")
                nc.vector.memset(vta[:, :, HD:HD + 1], 1.0)
                pv = ps_v[:, 0:C].rearrange("p (h d) -> p h d", h=NUM_HEADS)
                if v_bias:
                    nc.vector.tensor_tensor(
                        out=vta[:, :, 0:HD], in0=pv,
                        in1=cp[:, CP_VB:CP_VB + C].rearrange("p (h d) -> p h d", h=NUM_HEADS),
                        op=ALU.add)
                else:
                    nc.vector.tensor_copy(out=vta[:, :, 0:HD], in_=pv)
                vtas[st] = vta

        def emit_qk(b, ots):
            """q/k channel-major; bias+copy on ScalarE (idle in these phases)."""
            s = state[b]
            qks = s.setdefault("qks", [None] * QKT)
            for ot in ots:
                ps_q = ps_a.tile([128, HW], F32, tag="psa", name=f"psq_{b}_{ot}")
                for kt in range(CT):
                    for nh in range(NH):
                        nc.tensor.matmul(ps_q[:, nh * 512:(nh + 1) * 512],
                                         lhsT=wq_ap(kt, ot * 128, (ot + 1) * 128),
                                         rhs=s["xns"][kt][:, nh * 512:(nh + 1) * 512],
                                         start=(kt == 0), stop=(kt == CT - 1))
                qt = qkvpool.tile([128, HW], BF16, tag="qkv", name=f"qk_{b}_{ot}")
                if q_bias:
                    nc.vector.tensor_scalar_add(out=qt, in0=ps_q,
                                                scalar1=cp[:, CP_QKVB + ot:CP_QKVB + ot + 1])
                elif USE_QT_ACT:
                    nc.scalar.activation(out=qt, in_=ps_q, func=ACTF.Copy)
                else:
                    nc.vector.tensor_copy(out=qt, in_=ps_q)
                qks[ot] = qt

        def emit_pair(b, hp, last=False):
            """Head pair (2hp, 2hp+1).  Round-structured: PE queue per round is
            [S(st+1,j0) S(st+1,j1) AV(st,j0) AV(st,j1)]; exp j0 on ACT, j1 on
            DVE (Schraudolph)."""
            s = state[b]
            if "hts" not in s:
                s["hts"] = [hpool.tile([128, HW], BF16, tag="hm", name=f"hm_{b}_{i}")
                            for i in range(CT)]
            qt2 = s["qks"][hp]
            kt2 = s["qks"][CT + hp]
            vtas = s["vtas"]
            ps_os = [ps_av.tile([128, HW], F32, tag="psav", name=f"pso_{b}_{hp}_{j}")
                     for j in range(2)]
            ps_ss = {}

            def s_mm(st):
                # both heads' S tiles, nh-interleaved so the row-group-packed
                # (j0, j1) matmuls are queue-adjacent and stream concurrently
                ts = [ps_a.tile([128, HW], F32, tag="psa",
                                name=f"pss_{b}_{hp}_{st}_{j}") for j in range(2)]
                for nh in range(NH):
                    for j in range(2):
                        p0 = j * 64
                        nc.tensor.matmul(ts[j][:, nh * 512:(nh + 1) * 512],
                                         lhsT=kt2[p0:p0 + 64, st * 128:(st + 1) * 128],
                                         rhs=qt2[p0:p0 + 64, nh * 512:(nh + 1) * 512],
                                         start=True, stop=True,
                                         tile_position=(p0, 0) if USE_TP else None)
                for j in range(2):
                    ps_ss[(st, j)] = ts[j]

            def emit_exp(st, j):
                pexp = exppool.tile([128, HW], BF16, tag="pexp",
                                    name=f"pexp_{b}_{hp}_{st}_{j}")
                if USE_SCHR and j == 1:
                    nc.vector.tensor_scalar(out=pexp.bitcast(I16), in0=ps_ss[(st, j)],
                                            scalar1=SCHR_A, scalar2=SCHR_B,
                                            op0=ALU.mult, op1=ALU.add)
                else:
                    nc.scalar.activation(out=pexp, in_=ps_ss[(st, j)], func=ACTF.Exp,
                                         scale=1.0 / np.sqrt(HD))
                return pexp

            s_mm(0)
            for st in range(ST):
                pexps = [emit_exp(st, j) for j in range(2)]
                if st + 1 < ST:
                    s_mm(st + 1)
                for j in range(2):
                    h = 2 * hp + j
                    for nh in range(NH):
                        nc.tensor.matmul(ps_os[j][0:HD + 1, nh * 512:(nh + 1) * 512],
                                         lhsT=vtas[st][:, h, :],
                                         rhs=pexps[j][:, nh * 512:(nh + 1) * 512],
                                         start=(st == 0), stop=(st == ST - 1))

            # drain PSUM (frees AV banks for the next pair): hu + r rows on
            # ScalarE.  1/r via approx-fast reciprocal directly on the [1,1024]
            # rows (no DMA transpose -- a gather/scatter here costs ~2048
            # per-element descriptors = 10-18us of DMA queue time).  Broadcast
            # across partitions via a contiguous DRAM roundtrip.  The hts
            # multiplies are deferred into the next pair's emission so the
            # consumer FIFO never head-of-line blocks on the rb DMA.
            hus, ris = [], []
            for j in range(2):
                hu = hupool.tile([HD, HW], BF16, tag="hu", name=f"hu_{b}_{hp}_{j}")
                nc.scalar.activation(out=hu, in_=ps_os[j][0:HD, :], func=ACTF.Copy)
                hus.append(hu)
            for j in range(2):
                r1 = rsm.tile([1, HW], F32, tag=f"r2{j}", name=f"r2_{b}_{hp}_{j}")
                nc.scalar.activation(out=r1, in_=ps_os[j][HD:HD + 1, :],
                                     func=ACTF.Copy)
                ri = rsm.tile([1, HW], F32, tag=f"ri{j}", name=f"ri_{b}_{hp}_{j}")
                nc.vector.reciprocal_approx_fast(out=ri, in_=r1)
                ris.append(ri)
            if last:
                # tail fast path: broadcast 1/r across partitions with a K=1
                # matmul into rows 64:128 of the (now drained) AV psum, then
                # multiply on DVE.  Skips the DRAM roundtrip latency.
                for j in range(2):
                    for nh in range(NH):
                        nc.tensor.matmul(ps_os[j][HD:128, nh * 512:(nh + 1) * 512],
                                         lhsT=ones64, rhs=ris[j][:, nh * 512:(nh + 1) * 512],
                                         start=True, stop=True)
                    nc.vector.tensor_tensor(out=s["hts"][hp][j * 64:j * 64 + 64, :],
                                            in0=hus[j], in1=ps_os[j][HD:128, :],
                                            op=ALU.mult)
                return None
            rs = drams.tile([2, HW], F32, tag="rs", name=f"rs_{b}_{hp}")
            for j in range(2):
                nc.sync.dma_start(out=rs[j:j + 1, :], in_=ris[j])
            rb = rbpool.tile([HD, 2, HW], F32, tag="rb", name=f"rb_{b}_{hp}")
            rs_bc = bass.AP(tensor=rs.tensor, offset=rs.offset,
                            ap=[[0, HD]] + list(rs.ap))
            nc.sync.dma_start(out=rb, in_=rs_bc)

            eng = nc.gpsimd if USE_GP else nc.vector

            def finish():
                for j in range(2):
                    eng.tensor_tensor(out=s["hts"][hp][j * 64:j * 64 + 64, :],
                                      in0=hus[j], in1=rb[:, j, :], op=ALU.mult)
            return finish

        def emit_proj(b, ots):
            s = state[b]
            for ot in ots:
                ps_p = ps_a.tile([128, HW], F32, tag="psa", name=f"psp_{b}_{ot}")
                for kt in range(CT):
                    for nh in range(NH):
                        nc.tensor.matmul(ps_p[:, nh * 512:(nh + 1) * 512],
                                         lhsT=wproj_ap(kt, ot),
                                         rhs=s["hts"][kt][:, nh * 512:(nh + 1) * 512],
                                         start=(kt == 0), stop=(kt == CT - 1))
                yt = ypool.tile([128, HW], F32, tag="yt", name=f"yt_{b}_{ot}")
                if p_bias:
                    nc.vector.tensor_scalar_add(out=yt, in0=ps_p,
                                                scalar1=cp[:, CP_PROJB + ot:CP_PROJB + ot + 1])
                    nc.vector.tensor_tensor(out=yt, in0=yt, in1=x_sb[b][:, ot, :], op=ALU.add)
                else:
                    nc.vector.tensor_tensor(out=yt, in0=ps_p, in1=x_sb[b][:, ot, :], op=ALU.add)
                nc.sync.dma_start(out=out_d[b, ot * 128:(ot + 1) * 128, :], in_=yt)

        # ---- schedule ----
        emit_gn(0)
        emit_vt(0, range(ST))
        emit_qk(0, range(QKT))
        emit_gn(1)
        f00 = emit_pair(0, 0)
        emit_vt(1, range(0, 4))
        f01 = emit_pair(0, 1)
        f00()
        emit_vt(1, range(4, ST))
        f02 = emit_pair(0, 2)
        f01()
        emit_qk(1, [0, 4])
        f03 = emit_pair(0, 3)
        f02()
        emit_qk(1, [1, 5, 2, 6])
        f10 = emit_pair(1, 0)
        f03()
        emit_qk(1, [3, 7])
        emit_proj(0, [0, 1])
        f11 = emit_pair(1, 1)
        f10()
        emit_proj(0, [2, 3])
        f12 = emit_pair(1, 2)
        f11()
        f13 = emit_pair(1, 3, last=True)
        f12()
        emit_proj(1, range(CT))


def make_host_inputs(x, gn_gamma, gn_beta, qkv_w, qkv_b, proj_w, proj_b):
    """Full inputs -> list of per-core in_maps (packed weight/const tensors)."""
    x = np.asarray(x, dtype=np.float32).reshape(B, C, HW)
    wqkvT = np.asarray(qkv_w, dtype=np.float32).T          # [C, 3C]
    wprojT = np.asarray(proj_w, dtype=np.float32).T        # [C, C]
    wpack = np.zeros((128, CT, WPACK_COLS), dtype=ml_dtypes.bfloat16)
    for kt in range(CT):
        wpack[:, kt, :3 * C] = wqkvT[kt * 128:(kt + 1) * 128, :].astype(ml_dtypes.bfloat16)
        wpack[:, kt, WP_OFF:] = wprojT[kt * 128:(kt + 1) * 128, :].astype(ml_dtypes.bfloat16)

    cpack = np.zeros((128, CP_COLS), dtype=np.float32)
    for t in range(CT):
        for k in range(128):
            cpack[k, CP_GM + t * 32 + (t * 128 + k) // GS] = 1.0
            cpack[(t * 128 + k) // GS, CP_EM + t * 128 + k] = 1.0
    qkv_b = np.asarray(qkv_b, dtype=np.float32)
    for ot in range(QKT):
        cpack[:, CP_QKVB + ot] = qkv_b[ot * 128:(ot + 1) * 128]
    for t in range(CT):
        cpack[:, CP_PROJB + t] = np.asarray(proj_b, dtype=np.float32)[t * 128:(t + 1) * 128]
        cpack[:, CP_GNG + t] = np.asarray(gn_gamma, dtype=np.float32)[t * 128:(t + 1) * 128]
        cpack[:, CP_GNB + t] = np.asarray(gn_beta, dtype=np.float32)[t * 128:(t + 1) * 128]
    cpack[:, CP_VB:CP_VB + C] = qkv_b[2 * C:3 * C][None, :]

    shared = {"wpack": wpack, "cpack": cpack}
    return [dict(shared, x=np.ascontiguousarray(x[i * BPC:(i + 1) * BPC]))
            for i in range(NCORES)]


_NC_CACHE = {}


def _get_nc(q_bias=False, v_bias=False, p_bias=False):
    key = (q_bias, v_bias, p_bias)
    if key not in _NC_CACHE:
        _NC_CACHE[key] = build(q_bias=q_bias, v_bias=v_bias, p_bias=p_bias)
    return _NC_CACHE[key]


def kernel(x, gn_gamma, gn_beta, qkv_w, qkv_b, proj_w, proj_b):
    from concourse.bass_utils import run_bass_kernel_spmd
    qkv_b = np.asarray(qkv_b)
    nc = _get_nc(q_bias=bool(np.any(qkv_b[:2 * C])),
                 v_bias=bool(np.any(qkv_b[2 * C:])),
                 p_bias=bool(np.any(np.asarray(proj_b))))
    in_maps = make_host_inputs(x, gn_gamma, gn_beta, qkv_w, qkv_b, proj_w, proj_b)
    res = run_bass_kernel_spmd(nc, in_maps, list(range(NCORES)))
    out = np.concatenate([res.results[i]["out"] for i in range(NCORES)], axis=0)
    return out.reshape(B, C, H, W).astype(np.float32)


# revision 17
# speedup vs baseline: 1.2412x; 1.2412x over previous
"""AttentionBlock (GroupNorm -> qkv -> MHA -> proj -> residual) on 8 trn2 cores.

Data-parallel over batch: 16 batches -> 2 per core. No collectives.

Per-core math (per batch item, c=512 channels, hw=1024 spatial, 8 heads x 64):
  xn = groupnorm(x)                     [c, hw] layout (c on partitions)
  q,k = Wqk^T.T @ xn + b                [2c, hw]
  vT  = xn.T @ WvT + bv (broadcast)     [hw, c]   (direct transposed matmul)
  per head: S^T = k^T q                 [s=hw, t=hw]   (d=64 contraction)
            P = exp(S^T / 8)            (softmax w/o max-sub; logits ~N(0,1))
            AV: lhsT=[vT_h | ones] -> rows 0..64 unnormalized out, row 64 = r
            h = AV[0:64] * (1/r)
  y = x + WprojT.T @ h + proj_b

Rev B over the 417us baseline:
  - All startup DMAs batched (1 for x per batch, 1 packed const, 1 packed
    weight) -- the old kernel spent ~37us just issuing ~60 dma_starts.
  - softmax exp split across engines: head j=0 of each pair on ScalarE
    (table exp), j=1 on VectorE via a one-op Schraudolph exp that writes
    bf16 *bit patterns* through an int16 convert (err ~2% on pexp, which
    mostly cancels in p/r; ~1e-3 on the final output).  ACT was a 147us
    serial bottleneck; now ~93us ACT / ~85us DVE run in parallel under the
    PE's ~139us.
  - q/k bias+copy moved to ScalarE (idle during non-pair phases).
  - PSUM->SBUF r/hu drains on the otherwise idle GpSimd engine.
  - pair rounds emit PE work as [S(st+1,j0) S(st+1,j1) AV(st,j0) AV(st,j1)]
    so the PE queue never head-of-line blocks on the exp chain; qk/vt/proj
    fill the inter-pair gaps (PSUM: 2x[128,1024] S slots + 2 AV accums = 8
    banks, so fillers can't run inside rounds).
  - normalize: per-pair (not per-head) DMA transpose of r, reciprocal on 128
    lanes, bf16 broadcast (256KB/pair instead of 512KB/head).
"""

import os

import numpy as np
import ml_dtypes

import concourse.bass as bass
import concourse.tile as tile
import concourse.mybir as mybir
from concourse import bacc

NUM_HEADS = 8
NUM_GROUPS = 32
EPS = 1e-5
B, C, H, W = 16, 512, 32, 32
HW = H * W                  # 1024
NCORES = 8
BPC = B // NCORES           # 2 batches per core
HD = C // NUM_HEADS         # 64
GS = C // NUM_GROUPS        # 16 channels per group
CT = C // 128               # 4 channel tiles
QKT = 2 * C // 128          # 8 q+k output tiles
ST = HW // 128              # 8 sequence tiles
NH = HW // 512              # 2 moving-dim chunks of 512

F32 = mybir.dt.float32
BF16 = mybir.dt.bfloat16
F8E4 = mybir.dt.float8e4
F8E5 = mybir.dt.float8e5
I16 = mybir.dt.int16
I8 = mybir.dt.int8
U32 = mybir.dt.uint32
ALU = mybir.AluOpType
ACTF = mybir.ActivationFunctionType

USE_TP = os.environ.get("KERNEL_NO_TP") != "1"
# softmax exp for head j=1 of each pair on DVE (Schraudolph) instead of ACT
USE_SCHR = os.environ.get("KERNEL_NO_SCHR") != "1"
# r/hu PSUM drains on gpsimd
USE_GP = os.environ.get("KERNEL_NO_GP") != "1"
# q/k bias+copy on ScalarE
USE_QT_ACT = os.environ.get("KERNEL_QT_ACT") == "1"
# fp8 DoubleRow AV (vta e4m3, pexp e5m2, K=256 per matmul)
USE_DR = os.environ.get("KERNEL_NO_DR") != "1"

# Schraudolph exp in bf16-bit space: bits = int16(A*x + Bc); bf16 = bits
# computes exp(x/8) for raw logits x.  A = 2^7/(8 ln2).  C=5.0 tuned for
# round-to-nearest convert (max rel ~3.6%, mean +1.3%).
SCHR_A = 128.0 / (8.0 * np.log(2.0))
SCHR_B = 127.0 * 128.0 - 5.0
# e5m2 variant for the fp8 DoubleRow AV path
SCHR8_A = 4.0 / (8.0 * np.log(2.0))
SCHR8_B = 15.0 * 4.0 - 0.2

# packed const layout (f32 columns)
CP_GM = 0            # gm: [128, 4*32]   (group one-hot, per ct)
CP_QKVB = 128        # qkvb: [128, 8]    (bias col per qk out tile)
CP_PROJB = 136       # projb: [128, 4]
CP_GNG = 140         # gng: [128, 4]
CP_GNB = 144         # gnb: [128, 4]
CP_VB = 148          # vbias broadcast: [128, 8*64]
CP_EM = 660          # em: [32, 4*128]   (rows 0:32; transpose of gm)
CP_COLS = 660 + 4 * 128

WQK_W = 3 * C        # 1536 qkv cols per kt chunk
WP_OFF = 3 * C       # proj cols start
WPACK_COLS = 3 * C + C  # 2048


def build(num_devices=NCORES, q_bias=False, v_bias=False, p_bias=False):
    nc = bacc.Bacc("TRN2", target_bir_lowering=False, debug=False,
                   num_devices=num_devices)

    x_d = nc.dram_tensor("x", [BPC, C, HW], F32, kind="ExternalInput").ap()
    wpack_d = nc.dram_tensor("wpack", [128, CT, WPACK_COLS], BF16,
                             kind="ExternalInput").ap()
    cpack_d = nc.dram_tensor("cpack", [128, CP_COLS], F32,
                             kind="ExternalInput").ap()
    out_d = nc.dram_tensor("out", [BPC, C, HW], F32, kind="ExternalOutput").ap()

    with tile.TileContext(nc) as tc:
        _body(tc, nc, x_d, wpack_d, cpack_d, out_d, q_bias, v_bias, p_bias)
    nc.compile()
    return nc


def _body(tc, nc, x_d, wpack_d, cpack_d, out_d, q_bias, v_bias, p_bias):
    from contextlib import ExitStack
    ctx = ExitStack()
    with ctx:
        const = ctx.enter_context(tc.tile_pool(name="const", bufs=1))
        xpool = ctx.enter_context(tc.tile_pool(name="xpool", bufs=2))
        xnpool = ctx.enter_context(tc.tile_pool(name="xnpool", bufs=2 * CT))
        qkvpool = ctx.enter_context(tc.tile_pool(name="qkvpool", bufs=2 * QKT))
        vtapool = ctx.enter_context(tc.tile_pool(name="vtapool", bufs=2 * ST))
        exppool = ctx.enter_context(tc.tile_pool(name="exppool", bufs=6))
        hpool = ctx.enter_context(tc.tile_pool(name="hpool", bufs=2 * CT))
        hupool = ctx.enter_context(tc.tile_pool(name="hupool", bufs=4))
        rbpool = ctx.enter_context(tc.tile_pool(name="rbpool", bufs=2))
        ypool = ctx.enter_context(tc.tile_pool(name="ypool", bufs=2))
        smalls = ctx.enter_context(tc.tile_pool(name="smalls", bufs=6))
        rsm = ctx.enter_context(tc.tile_pool(name="rsm", bufs=1))
        drams = ctx.enter_context(tc.tile_pool(name="drams", bufs=4, space="DRAM"))
        ps_a = ctx.enter_context(tc.tile_pool(name="ps_a", bufs=2, space="PSUM"))
        ps_av = ctx.enter_context(tc.tile_pool(name="ps_av", bufs=2, space="PSUM"))

        # ---- batched input DMAs (4 dma_starts total) ----
        x_sb = [xpool.tile([128, CT, HW], F32, tag="x", name=f"x_{b}")
                for b in range(BPC)]
        cp = const.tile([128, CP_COLS], F32)
        wp = const.tile([128, CT, WPACK_COLS], BF16)

        def dma_x(b):
            # per-ct chunks so gn(b) statistics start on the first 512KB
            for ct in range(CT):
                nc.sync.dma_start(out=x_sb[b][:, ct, :],
                                  in_=x_d[b, ct * 128:(ct + 1) * 128, :])

        nc.sync.dma_start(out=cp, in_=cpack_d)
        dma_x(0)
        # v-cols first (vt runs first), then q/k, then proj
        nc.sync.dma_start(out=wp[:, :, 2 * C:3 * C], in_=wpack_d[:, :, 2 * C:3 * C])
        nc.sync.dma_start(out=wp[:, :, 0:2 * C], in_=wpack_d[:, :, 0:2 * C])
        nc.sync.dma_start(out=wp[:, :, WP_OFF:], in_=wpack_d[:, :, WP_OFF:])
        dma_x(1)

        magic = const.tile([NUM_GROUPS, 1], U32)
        nc.vector.memset(magic, 0x5F3759DF)
        ones64 = const.tile([1, HD], F32)
        nc.vector.memset(ones64, 1.0)

        def wq_ap(kt, c0, c1):
            return wp[:, kt, c0:c1]

        def wproj_ap(kt, ot):
            return wp[:, kt, WP_OFF + ot * 128:WP_OFF + (ot + 1) * 128]

        state = [dict() for _ in range(BPC)]

        def emit_gn(b):
            """group-norm of x_sb[b] -> xns (bf16).  rstd via DVE quake+Newton
            so ScalarE only ever holds the exp table set."""
            s = state[b]
            cm2s, xns = [], []
            for ct in range(CT):
                stats = smalls.tile([128, 2, 6], F32, tag="bnst", name=f"bnst_{b}_{ct}")
                for sg in range(2):
                    nc.vector.bn_stats(out=stats[:, sg, :],
                                       in_=x_sb[b][:, ct, sg * 512:(sg + 1) * 512])
                cmv = smalls.tile([128, 2], F32, tag="cmv", name=f"cmv_{b}_{ct}")
                nc.vector.bn_aggr(out=cmv, in_=stats)
                cm2 = smalls.tile([128, 2], F32, tag="cm2", name=f"cm2_{b}_{ct}")
                nc.vector.tensor_copy(out=cm2[:, 0:1], in_=cmv[:, 0:1])
                nc.vector.tensor_tensor(out=cm2[:, 1:2], in0=cmv[:, 0:1], in1=cmv[:, 0:1], op=ALU.mult)
                nc.vector.tensor_tensor(out=cm2[:, 1:2], in0=cm2[:, 1:2], in1=cmv[:, 1:2], op=ALU.add)
                cm2s.append(cm2)
                xn = xnpool.tile([128, HW], BF16, tag="xn", name=f"xn_{b}_{ct}")
                xns.append(xn)
            ps_g = ps_a.tile([128, HW], F32, tag="psa", name=f"psg_{b}")
            for ct in range(CT):
                nc.tensor.matmul(ps_g[0:NUM_GROUPS, 0:2],
                                 lhsT=cp[:, CP_GM + ct * 32:CP_GM + (ct + 1) * 32],
                                 rhs=cm2s[ct], start=(ct == 0), stop=(ct == CT - 1))
            gstat = smalls.tile([NUM_GROUPS, 2], F32, tag="gstat", name=f"gstat_{b}")
            nc.vector.tensor_scalar_mul(out=gstat, in0=ps_g[0:NUM_GROUPS, 0:2], scalar1=1.0 / GS)
            var_g = smalls.tile([NUM_GROUPS, 1], F32, tag="varg", name=f"varg_{b}")
            nc.vector.tensor_tensor(out=var_g, in0=gstat[:, 0:1], in1=gstat[:, 0:1], op=ALU.mult)
            nc.vector.tensor_tensor(out=var_g, in0=gstat[:, 1:2], in1=var_g, op=ALU.subtract)
            nc.vector.tensor_scalar_add(out=var_g, in0=var_g, scalar1=EPS)
            y_n = smalls.tile([NUM_GROUPS, 1], F32, tag="yn", name=f"yn_{b}")
            t_n = smalls.tile([NUM_GROUPS, 1], F32, tag="tn", name=f"tn_{b}")
            nc.vector.tensor_scalar(out=y_n.bitcast(U32), in0=var_g.bitcast(U32),
                                    scalar1=1, scalar2=None, op0=ALU.logical_shift_right)
            nc.vector.tensor_tensor(out=y_n.bitcast(U32), in0=magic,
                                    in1=y_n.bitcast(U32), op=ALU.subtract)
            for _ in range(3):
                nc.vector.tensor_tensor(out=t_n, in0=var_g, in1=y_n, op=ALU.mult)
                nc.vector.tensor_tensor(out=t_n, in0=t_n, in1=y_n, op=ALU.mult)
                nc.vector.tensor_scalar(out=t_n, in0=t_n, scalar1=-0.5, scalar2=1.5,
                                        op0=ALU.mult, op1=ALU.add)
                nc.vector.tensor_tensor(out=y_n, in0=y_n, in1=t_n, op=ALU.mult)
            nc.vector.tensor_copy(out=gstat[:, 1:2], in_=y_n)
            for ct in range(CT):
                ps_e = ps_a.tile([128, HW], F32, tag="psa", name=f"pse_{b}_{ct}")
                nc.tensor.matmul(ps_e[:, 0:2],
                                 lhsT=cp[0:32, CP_EM + ct * 128:CP_EM + (ct + 1) * 128],
                                 rhs=gstat, start=True, stop=True)
                sc = smalls.tile([128, 1], F32, tag="sc", name=f"sc_{b}_{ct}")
                bi = smalls.tile([128, 1], F32, tag="bi", name=f"bi_{b}_{ct}")
                nc.vector.tensor_tensor(out=sc, in0=cp[:, CP_GNG + ct:CP_GNG + ct + 1],
                                        in1=ps_e[:, 1:2], op=ALU.mult)
                nc.vector.tensor_tensor(out=bi, in0=ps_e[:, 0:1], in1=sc, op=ALU.mult)
                nc.vector.tensor_tensor(out=bi, in0=cp[:, CP_GNB + ct:CP_GNB + ct + 1],
                                        in1=bi, op=ALU.subtract)
                xeng = nc.gpsimd if USE_GP else nc.vector
                xeng.tensor_scalar(out=xns[ct], in0=x_sb[b][:, ct, :],
                                   scalar1=sc, scalar2=bi, op0=ALU.mult, op1=ALU.add)
            s["xns"] = xns

        def emit_vt(b, sts):
            """vT[s, o] = sum_c xn[c, s] * WvT[c, o]; +bias col65=1 for rowsum."""
            s = state[b]
            vtas = s.setdefault("vtas", [None] * ST)
            for st in sts:
                ps_v = ps_a.tile([128, C], F32, tag="psa", name=f"psv_{b}_{st}")
                for kt in range(CT):
                    nc.tensor.matmul(ps_v[:, 0:C],
                                     lhsT=s["xns"][kt][:, st * 128:(st + 1) * 128],
                                     rhs=wq_ap(kt, 2 * C, 3 * C),
                                     start=(kt == 0), stop=(kt == CT - 1))
                pv = ps_v[:, 0:C].rearrange("p (h d) -> p h d", h=NUM_HEADS)
                if USE_DR:
                    vtaps = s.setdefault("vtaps", [None] * (ST // 2))
                    sp, k = st // 2, st % 2
                    if vtaps[sp] is None:
                        # head stride 66 (528B) keeps the DoubleRow dual-
                        # subtile step 16B-aligned (s3_lw_dual_fp8_restrictions)
                        vtaps[sp] = vtapool.tile([128, 2, NUM_HEADS, HD + 2], F8E4,
                                                 tag="vta", name=f"vta_{b}_{sp}")
                        nc.vector.memset(vtaps[sp][:, :, :, HD:HD + 1], 1.0)
                    dst = vtaps[sp][:, k, :, 0:HD]
                else:
                    vta = vtapool.tile([128, NUM_HEADS, HD + 1], BF16, tag="vta",
                                       name=f"vta_{b}_{st}")
                    nc.vector.memset(vta[:, :, HD:HD + 1], 1.0)
                    vtas[st] = vta
                    dst = vta[:, :, 0:HD]
                if v_bias:
                    nc.vector.tensor_tensor(
                        out=dst, in0=pv,
                        in1=cp[:, CP_VB:CP_VB + C].rearrange("p (h d) -> p h d", h=NUM_HEADS),
                        op=ALU.add)
                else:
                    nc.vector.tensor_copy(out=dst, in_=pv)

        def emit_qk(b, ots):
            """q/k channel-major; bias+copy on ScalarE (idle in these phases)."""
            s = state[b]
            qks = s.setdefault("qks", [None] * QKT)
            for ot in ots:
                ps_q = ps_a.tile([128, HW], F32, tag="psa", name=f"psq_{b}_{ot}")
                for kt in range(CT):
                    for nh in range(NH):
                        nc.tensor.matmul(ps_q[:, nh * 512:(nh + 1) * 512],
                                         lhsT=wq_ap(kt, ot * 128, (ot + 1) * 128),
                                         rhs=s["xns"][kt][:, nh * 512:(nh + 1) * 512],
                                         start=(kt == 0), stop=(kt == CT - 1))
                qt = qkvpool.tile([128, HW], BF16, tag="qkv", name=f"qk_{b}_{ot}")
                if q_bias:
                    nc.vector.tensor_scalar_add(out=qt, in0=ps_q,
                                                scalar1=cp[:, CP_QKVB + ot:CP_QKVB + ot + 1])
                elif USE_QT_ACT:
                    nc.scalar.activation(out=qt, in_=ps_q, func=ACTF.Copy)
                else:
                    nc.vector.tensor_copy(out=qt, in_=ps_q)
                qks[ot] = qt

        def emit_pair(b, hp, last=False):
            """Head pair (2hp, 2hp+1).  Round-structured: PE queue per round is
            [S(st+1,j0) S(st+1,j1) AV(st,j0) AV(st,j1)]; exp j0 on ACT, j1 on
            DVE (Schraudolph)."""
            s = state[b]
            if "hts" not in s:
                s["hts"] = [hpool.tile([128, HW], BF16, tag="hm", name=f"hm_{b}_{i}")
                            for i in range(CT)]
            qt2 = s["qks"][hp]
            kt2 = s["qks"][CT + hp]
            vtas = s.get("vtas")
            vtaps = s.get("vtaps")
            ps_os = [ps_av.tile([128, HW], F32, tag="psav", name=f"pso_{b}_{hp}_{j}")
                     for j in range(2)]
            ps_ss = {}

            def s_mm(st):
                # both heads' S tiles, nh-interleaved so the row-group-packed
                # (j0, j1) matmuls are queue-adjacent and stream concurrently
                ts = [ps_a.tile([128, HW], F32, tag="psa",
                                name=f"pss_{b}_{hp}_{st}_{j}") for j in range(2)]
                for nh in range(NH):
                    for j in range(2):
                        p0 = j * 64
                        nc.tensor.matmul(ts[j][:, nh * 512:(nh + 1) * 512],
                                         lhsT=kt2[p0:p0 + 64, st * 128:(st + 1) * 128],
                                         rhs=qt2[p0:p0 + 64, nh * 512:(nh + 1) * 512],
                                         start=True, stop=True,
                                         tile_position=(p0, 0) if USE_TP else None)
                for j in range(2):
                    ps_ss[(st, j)] = ts[j]

            pexp_pairs = {}

            def emit_exp(st, j):
                if USE_DR:
                    sp, k = st // 2, st % 2
                    if k == 0:
                        pexp_pairs[(sp, j)] = exppool.tile(
                            [128, 2, HW], F8E5, tag="pexp",
                            name=f"pexp_{b}_{hp}_{sp}_{j}")
                    dst = pexp_pairs[(sp, j)][:, k, :]
                    if USE_SCHR and j == 1:
                        nc.vector.tensor_scalar(out=dst.bitcast(I8), in0=ps_ss[(st, j)],
                                                scalar1=SCHR8_A, scalar2=SCHR8_B,
                                                op0=ALU.mult, op1=ALU.add)
                    else:
                        nc.scalar.activation(out=dst, in_=ps_ss[(st, j)], func=ACTF.Exp,
                                             scale=1.0 / np.sqrt(HD))
                    return None
                pexp = exppool.tile([128, HW], BF16, tag="pexp",
                                    name=f"pexp_{b}_{hp}_{st}_{j}")
                if USE_SCHR and j == 1:
                    nc.vector.tensor_scalar(out=pexp.bitcast(I16), in0=ps_ss[(st, j)],
                                            scalar1=SCHR_A, scalar2=SCHR_B,
                                            op0=ALU.mult, op1=ALU.add)
                else:
                    nc.scalar.activation(out=pexp, in_=ps_ss[(st, j)], func=ACTF.Exp,
                                         scale=1.0 / np.sqrt(HD))
                return pexp

            s_mm(0)
            for st in range(ST):
                pexps = [emit_exp(st, j) for j in range(2)]
                if st + 1 < ST:
                    s_mm(st + 1)
                if USE_DR:
                    if st % 2 == 1:
                        sp = st // 2
                        for j in range(2):
                            h = 2 * hp + j
                            for nh in range(NH):
                                nc.tensor.matmul(
                                    ps_os[j][0:HD + 1, nh * 512:(nh + 1) * 512],
                                    lhsT=vtaps[sp][:, :, h, 0:HD + 1],
                                    rhs=pexp_pairs[(sp, j)][:, :, nh * 512:(nh + 1) * 512],
                                    start=(sp == 0), stop=(sp == ST // 2 - 1),
                                    perf_mode=mybir.MatmulPerfMode.DoubleRow)
                    continue
                for j in range(2):
                    h = 2 * hp + j
                    for nh in range(NH):
                        nc.tensor.matmul(ps_os[j][0:HD + 1, nh * 512:(nh + 1) * 512],
                                         lhsT=vtas[st][:, h, :],
                                         rhs=pexps[j][:, nh * 512:(nh + 1) * 512],
                                         start=(st == 0), stop=(st == ST - 1))

            # drain PSUM (frees AV banks for the next pair): hu + r rows on
            # ScalarE.  1/r via approx-fast reciprocal directly on the [1,1024]
            # rows (no DMA transpose -- a gather/scatter here costs ~2048
            # per-element descriptors = 10-18us of DMA queue time).  Broadcast
            # across partitions via a contiguous DRAM roundtrip.  The hts
            # multiplies are deferred into the next pair's emission so the
            # consumer FIFO never head-of-line blocks on the rb DMA.
            hus, ris = [], []
            for j in range(2):
                hu = hupool.tile([HD, HW], BF16, tag="hu", name=f"hu_{b}_{hp}_{j}")
                nc.scalar.activation(out=hu, in_=ps_os[j][0:HD, :], func=ACTF.Copy)
                hus.append(hu)
            for j in range(2):
                r1 = rsm.tile([1, HW], F32, tag=f"r2{j}", name=f"r2_{b}_{hp}_{j}")
                nc.scalar.activation(out=r1, in_=ps_os[j][HD:HD + 1, :],
                                     func=ACTF.Copy)
                ri = rsm.tile([1, HW], F32, tag=f"ri{j}", name=f"ri_{b}_{hp}_{j}")
                nc.vector.reciprocal_approx_fast(out=ri, in_=r1)
                ris.append(ri)
            if last:
                # tail fast path: broadcast 1/r across partitions with a K=1
                # matmul into rows 64:128 of the (now drained) AV psum, then
                # multiply on DVE.  Skips the DRAM roundtrip latency.
                for j in range(2):
                    for nh in range(NH):
                        nc.tensor.matmul(ps_os[j][HD:128, nh * 512:(nh + 1) * 512],
                                         lhsT=ones64, rhs=ris[j][:, nh * 512:(nh + 1) * 512],
                                         start=True, stop=True)
                    nc.vector.tensor_tensor(out=s["hts"][hp][j * 64:j * 64 + 64, :],
                                            in0=hus[j], in1=ps_os[j][HD:128, :],
                                            op=ALU.mult)
                return None
            rs = drams.tile([2, HW], F32, tag="rs", name=f"rs_{b}_{hp}")
            for j in range(2):
                nc.sync.dma_start(out=rs[j:j + 1, :], in_=ris[j])
            rb = rbpool.tile([HD, 2, HW], F32, tag="rb", name=f"rb_{b}_{hp}")
            rs_bc = bass.AP(tensor=rs.tensor, offset=rs.offset,
                            ap=[[0, HD]] + list(rs.ap))
            nc.sync.dma_start(out=rb, in_=rs_bc)

            eng = nc.gpsimd if USE_GP else nc.vector

            def finish():
                for j in range(2):
                    eng.tensor_tensor(out=s["hts"][hp][j * 64:j * 64 + 64, :],
                                      in0=hus[j], in1=rb[:, j, :], op=ALU.mult)
            return finish

        def emit_proj(b, ots):
            s = state[b]
            for ot in ots:
                ps_p = ps_a.tile([128, HW], F32, tag="psa", name=f"psp_{b}_{ot}")
                for kt in range(CT):
                    for nh in range(NH):
                        nc.tensor.matmul(ps_p[:, nh * 512:(nh + 1) * 512],
                                         lhsT=wproj_ap(kt, ot),
                                         rhs=s["hts"][kt][:, nh * 512:(nh + 1) * 512],
                                         start=(kt == 0), stop=(kt == CT - 1))
                yt = ypool.tile([128, HW], F32, tag="yt", name=f"yt_{b}_{ot}")
                if p_bias:
                    nc.vector.tensor_scalar_add(out=yt, in0=ps_p,
                                                scalar1=cp[:, CP_PROJB + ot:CP_PROJB + ot + 1])
                    nc.vector.tensor_tensor(out=yt, in0=yt, in1=x_sb[b][:, ot, :], op=ALU.add)
                else:
                    nc.vector.tensor_tensor(out=yt, in0=ps_p, in1=x_sb[b][:, ot, :], op=ALU.add)
                nc.sync.dma_start(out=out_d[b, ot * 128:(ot + 1) * 128, :], in_=yt)

        # ---- schedule ----
        emit_gn(0)
        emit_vt(0, range(ST))
        emit_qk(0, range(QKT))
        emit_gn(1)
        f00 = emit_pair(0, 0)
        emit_vt(1, range(0, 4))
        f01 = emit_pair(0, 1)
        f00()
        emit_vt(1, range(4, ST))
        f02 = emit_pair(0, 2)
        f01()
        emit_qk(1, [0, 4])
        f03 = emit_pair(0, 3)
        f02()
        emit_qk(1, [1, 5, 2, 6])
        f10 = emit_pair(1, 0)
        f03()
        emit_qk(1, [3, 7])
        emit_proj(0, [0, 1])
        f11 = emit_pair(1, 1)
        f10()
        emit_proj(0, [2, 3])
        f12 = emit_pair(1, 2)
        f11()
        f13 = emit_pair(1, 3, last=True)
        f12()
        emit_proj(1, range(CT))


def make_host_inputs(x, gn_gamma, gn_beta, qkv_w, qkv_b, proj_w, proj_b):
    """Full inputs -> list of per-core in_maps (packed weight/const tensors)."""
    x = np.asarray(x, dtype=np.float32).reshape(B, C, HW)
    wqkvT = np.asarray(qkv_w, dtype=np.float32).T          # [C, 3C]
    wprojT = np.asarray(proj_w, dtype=np.float32).T        # [C, C]
    wpack = np.zeros((128, CT, WPACK_COLS), dtype=ml_dtypes.bfloat16)
    for kt in range(CT):
        wpack[:, kt, :3 * C] = wqkvT[kt * 128:(kt + 1) * 128, :].astype(ml_dtypes.bfloat16)
        wpack[:, kt, WP_OFF:] = wprojT[kt * 128:(kt + 1) * 128, :].astype(ml_dtypes.bfloat16)

    cpack = np.zeros((128, CP_COLS), dtype=np.float32)
    for t in range(CT):
        for k in range(128):
            cpack[k, CP_GM + t * 32 + (t * 128 + k) // GS] = 1.0
            cpack[(t * 128 + k) // GS, CP_EM + t * 128 + k] = 1.0
    qkv_b = np.asarray(qkv_b, dtype=np.float32)
    for ot in range(QKT):
        cpack[:, CP_QKVB + ot] = qkv_b[ot * 128:(ot + 1) * 128]
    for t in range(CT):
        cpack[:, CP_PROJB + t] = np.asarray(proj_b, dtype=np.float32)[t * 128:(t + 1) * 128]
        cpack[:, CP_GNG + t] = np.asarray(gn_gamma, dtype=np.float32)[t * 128:(t + 1) * 128]
        cpack[:, CP_GNB + t] = np.asarray(gn_beta, dtype=np.float32)[t * 128:(t + 1) * 128]
    cpack[:, CP_VB:CP_VB + C] = qkv_b[2 * C:3 * C][None, :]

    shared = {"wpack": wpack, "cpack": cpack}
    return [dict(shared, x=np.ascontiguousarray(x[i * BPC:(i + 1) * BPC]))
            for i in range(NCORES)]


_NC_CACHE = {}


def _get_nc(q_bias=False, v_bias=False, p_bias=False):
    key = (q_bias, v_bias, p_bias)
    if key not in _NC_CACHE:
        _NC_CACHE[key] = build(q_bias=q_bias, v_bias=v_bias, p_bias=p_bias)
    return _NC_CACHE[key]


def kernel(x, gn_gamma, gn_beta, qkv_w, qkv_b, proj_w, proj_b):
    from concourse.bass_utils import run_bass_kernel_spmd
    qkv_b = np.asarray(qkv_b)
    nc = _get_nc(q_bias=bool(np.any(qkv_b[:2 * C])),
                 v_bias=bool(np.any(qkv_b[2 * C:])),
                 p_bias=bool(np.any(np.asarray(proj_b))))
    in_maps = make_host_inputs(x, gn_gamma, gn_beta, qkv_w, qkv_b, proj_w, proj_b)
    res = run_bass_kernel_spmd(nc, in_maps, list(range(NCORES)))
    out = np.concatenate([res.results[i]["out"] for i in range(NCORES)], axis=0)
    return out.reshape(B, C, H, W).astype(np.float32)
